# revision 7
# baseline (speedup 1.0000x reference)
"""Masked-BCE valid-region loss on 8 Trainium2 NeuronCores.

Inputs (full): cancer_logits [32,1,512,512] f32, label [32] f32,
prostate_mask [32,1,512,512] f32, needle_mask [32,1,512,512] f32.
Output: scalar f32 loss.

Sharding: data-parallel over batch - 4 images per core. The host packs
all three tensors as fp8e4m3 (1 byte/pixel; the 2e-2 harness tolerance
dwarfs the rounding effect, and mask threshold flips hit numerator and
denominator on the same pixels so the ratio moves ~1e-4): 3 MB of HBM
traffic per core vs 12 MB in f32.

Math: with m = (min(p,n) > 0.5) and y constant per image,

    bce = softplus(x) - x*y,   softplus(z) = -ln(sigmoid(-z))
    sum_masked softplus(x) = -sum ln(sigmoid(-xm)) - (N - count)*ln2

since masked-out elements have xm = 0 and contribute sigmoid(0) = 1/2.
The ln of 1M products is folded as ln(prod) over groups of 8 via a
bf16 multiply tree (sigma <= 1 so products only shrink; worst case
8*6.55 = 52 lns stays far above bf16 underflow), so the ACT engine
runs ONE full-size pass (sigmoid) instead of exp+ln, and the final ln
touches N/8 elements.

Device pipeline per chunk:

    pb,nb,xb = cast-DMA fp8->bf16      # SWDGE casts during the DMA
    pb  = min(pb, nb)                  # DVE tensor_tensor, 2x at bf16
    m   = (pb > 0.5)                   # DVE tensor_scalar, 4x
    xm  = m * xb                       # DVE tensor_tensor, 2x
    s   = sigmoid(-xm)                 # ACT (the only full-size pass)
    s2/s4/s8 = halve-multiply tree     # DVE 2x, contiguous halves
    cnt += ones' @ m;  sxm_img += ones' @ xm   # TensorE -> PSUM
    ...all chunks done...
    ln(s8) with accum_out per chunk    # ACT, one table switch total

tensor_scalar cannot carry an accumulator (BIR verifier rejects it)
and Pool/GpSimd cannot run TensorScalarPtr at all, so count and the
per-image sum(x*m) ride TensorE ones-matmuls into PSUM banks. The act
tables are pinned to {sigmoid_and_others, natural_log} so exactly two
ACT_TABLE_LOADs are emitted.
"""

import sys

for _p in ("/opt/trn_rl_repo", "/root/.axon_site/_ro/trn_rl_repo"):
    if _p not in sys.path:
        sys.path.append(_p)

import ml_dtypes
import numpy as np

import concourse.bacc as bacc
import concourse.tile as tile
from concourse import mybir
from concourse.bass_utils import run_bass_kernel_spmd

B, H, W = 32, 512, 512
N_CORES = 8
IMGS_PER_CORE = B // N_CORES  # 4
P = 128
FD = (H * W) // P  # 2048 free-dim elements per partition per image
N_PER_IMG = H * W  # 262144
TOT_FD = IMGS_PER_CORE * FD  # 8192
# image-aligned chunks (multiples of 512 for PE blocks, of 8 for the tree);
# a split first image shortens the pipeline ramp.
CHUNK_FDS = [1024, 1024, 2048, 2048, 2048]
N_CHUNKS = len(CHUNK_FDS)
TREE_K = 8  # elements per ln group; 8*6.55 < 88 so bf16 never underflows

_nc_cache = None


def _patch_act_tables():
    """Keep only {sigmoid_and_others, natural_log} activation sets so the
    per-activation table picker emits exactly one ACT_TABLE_LOAD per set
    (sigmoid for the main pass, ln for the batched tail) instead of
    reloading alternating sets."""
    import concourse.hw_specs as hw_specs

    if getattr(bacc, "_act_tables_patched", False):
        return
    orig = hw_specs.get_activation_tables

    def patched(module_arch):
        tables = orig(module_arch)
        keep = ("sigmoid_and_others", "natural_log")
        return {
            name: (funcs if name in keep else set())
            for name, funcs in tables.items()
        }

    bacc.get_activation_tables = patched
    bacc._act_tables_patched = True


def _build_bass():
    _patch_act_tables()
    f32 = mybir.dt.float32
    bf16 = mybir.dt.bfloat16
    fp8 = mybir.dt.float8e4
    nc = bacc.Bacc()
    p_d = nc.dram_tensor("p8", [P, TOT_FD], fp8, kind="ExternalInput")
    n_d = nc.dram_tensor("n8", [P, TOT_FD], fp8, kind="ExternalInput")
    x_d = nc.dram_tensor("x8", [P, TOT_FD], fp8, kind="ExternalInput")
    # ln-accum column
    ln_o = nc.dram_tensor("lncols", [P, 1], f32, kind="ExternalOutput")
    # [count row | img0..img3 rows], 512 f32 each
    red_o = nc.dram_tensor("red", [1, 5 * 512], f32, kind="ExternalOutput")

    with tile.TileContext(nc) as tc:
        with (
            tc.tile_pool(name="io", bufs=3) as io_pool,
            tc.tile_pool(name="work", bufs=3) as work_pool,
            tc.tile_pool(name="keep", bufs=1) as keep_pool,
            tc.tile_pool(name="psum", bufs=1, space="PSUM") as psum_pool,
        ):
            ones1 = keep_pool.tile([P, 1], bf16)
            nc.vector.memset(ones1, 1.0)
            lncols = keep_pool.tile([P, 1], f32)
            # every chunk's tree lands in one shared tile, so the single Ln
            # op depends on ALL trees and cannot be scheduled between
            # sigmoids (which would thrash the two ACT table sets).
            s8all = keep_pool.tile([P, TOT_FD // TREE_K], bf16)
            cnt_ps = psum_pool.tile([1, 512], f32, tag="cnt")
            img_ps = [
                psum_pool.tile([1, 512], f32, tag=f"img{i}", name=f"img_ps{i}")
                for i in range(IMGS_PER_CORE)
            ]

            total_blocks = TOT_FD // 512  # count matmuls over all chunks
            blocks_per_img = FD // 512  # per-image xm matmuls
            cnt_done = 0
            img_done = [0] * IMGS_PER_CORE
            # one cast-DMA per image per tensor (image 0 split in two for
            # faster ramp); compute chunks slice these tiles.
            DMA_FDS = [1024, 1024, 2048, 2048, 2048]
            dma_tiles = {}  # global col -> (tile, base)
            doff = 0
            for d, dfd in enumerate(DMA_FDS):
                pb_t = io_pool.tile([P, dfd], bf16, tag=f"pb{d}", name=f"pb{d}")
                nc.gpsimd.dma_start(out=pb_t, in_=p_d[:, doff : doff + dfd])
                xb_t = io_pool.tile([P, dfd], bf16, tag=f"xb{d}", name=f"xb{d}")
                nc.gpsimd.dma_start(out=xb_t, in_=x_d[:, doff : doff + dfd])
                nb_t = io_pool.tile([P, dfd], bf16, tag=f"nb{d}", name=f"nb{d}")
                nc.gpsimd.dma_start(out=nb_t, in_=n_d[:, doff : doff + dfd])
                dma_tiles[doff] = (pb_t, nb_t, xb_t)
                doff += dfd

            off = 0
            for c, cfd in enumerate(CHUNK_FDS):
                pb, nb, xb = dma_tiles[off]

                # pt = min(p, n), in place over pb
                nc.vector.tensor_tensor(
                    out=pb, in0=pb, in1=nb, op=mybir.AluOpType.min
                )
                m = work_pool.tile([P, cfd], bf16, tag="m")
                nc.vector.tensor_scalar(
                    out=m, in0=pb, scalar1=0.5, scalar2=None,
                    op0=mybir.AluOpType.is_gt,
                )
                xm = work_pool.tile([P, cfd], bf16, tag="xm")
                nc.vector.tensor_tensor(
                    out=xm, in0=m, in1=xb, op=mybir.AluOpType.mult
                )
                # s = sigmoid(-xm); bf16 out feeds the 2x multiply tree
                s = work_pool.tile([P, cfd], bf16, tag="s")
                nc.scalar.activation(
                    out=s, in_=xm, func=mybir.ActivationFunctionType.Sigmoid,
                    scale=-1.0,
                )
                h = cfd // 2
                s2 = work_pool.tile([P, h], bf16, tag="s2")
                nc.vector.tensor_tensor(
                    out=s2, in0=s[:, :h], in1=s[:, h:], op=mybir.AluOpType.mult
                )
                q = cfd // 4
                s4 = work_pool.tile([P, q], bf16, tag="s4")
                nc.vector.tensor_tensor(
                    out=s4, in0=s2[:, :q], in1=s2[:, q:], op=mybir.AluOpType.mult
                )
                e = cfd // 8
                s8 = s8all[:, off // 8 : off // 8 + e]
                nc.vector.tensor_tensor(
                    out=s8, in0=s4[:, :e], in1=s4[:, e:], op=mybir.AluOpType.mult
                )

                # TensorE reductions: count (bank cnt) + per-image sum(xm)
                img = off // FD
                for s0 in range(0, cfd, 512):
                    nc.tensor.matmul(
                        cnt_ps, ones1, m[:, s0 : s0 + 512],
                        start=(cnt_done == 0),
                        stop=(cnt_done == total_blocks - 1),
                    )
                    cnt_done += 1
                    i = (off + s0) // FD
                    nc.tensor.matmul(
                        img_ps[i], ones1, xm[:, s0 : s0 + 512],
                        start=(img_done[i] == 0),
                        stop=(img_done[i] == blocks_per_img - 1),
                    )
                    img_done[i] += 1
                off += cfd

            # one ln over all trees: one table switch, one accum read
            lnv = keep_pool.tile([P, TOT_FD // TREE_K], bf16)
            nc.scalar.activation(
                out=lnv, in_=s8all, func=mybir.ActivationFunctionType.Ln,
                accum_out=lncols[:, 0:1],
            )

            red_sb = keep_pool.tile([1, 5 * 512], f32)
            nc.vector.tensor_scalar_add(out=red_sb[:, 0:512], in0=cnt_ps, scalar1=0.0)
            for i in range(IMGS_PER_CORE):
                nc.vector.tensor_scalar_add(
                    out=red_sb[:, (i + 1) * 512 : (i + 2) * 512],
                    in0=img_ps[i], scalar1=0.0,
                )
            nc.sync.dma_start(out=ln_o[:], in_=lncols)
            nc.sync.dma_start(out=red_o[:], in_=red_sb)
    nc.finalize()
    return nc


def _get_nc():
    global _nc_cache
    if _nc_cache is None:
        _nc_cache = _build_bass()
    return _nc_cache


def _make_in_maps(cancer_logits, prostate_mask, needle_mask):
    f8 = ml_dtypes.float8_e4m3
    # [B,1,H,W] -> [CORE, P, IMG*FD] image-major flat per-partition streams
    def pack(a):
        a = np.asarray(a, dtype=np.float32).reshape(B, P, FD).astype(f8)
        a = a.reshape(N_CORES, IMGS_PER_CORE, P, FD).transpose(0, 2, 1, 3)
        return np.ascontiguousarray(a).reshape(N_CORES, P, TOT_FD)

    x8 = pack(cancer_logits)
    p8 = pack(prostate_mask)
    n8 = pack(needle_mask)
    return [
        {"p8": p8[c], "n8": n8[c], "x8": x8[c]} for c in range(N_CORES)
    ]


def _combine(results, label):
    y = np.asarray(label, dtype=np.float64).reshape(B)
    ln2 = float(np.log(2.0))
    n_core = IMGS_PER_CORE * N_PER_IMG
    num = 0.0
    cnt = 0.0
    for c in range(N_CORES):
        red = np.asarray(results[c]["red"], dtype=np.float64).reshape(5 * 512)
        count = red[:512].sum()
        sxm = red[512:].reshape(IMGS_PER_CORE, 512).sum(axis=1)
        lns = np.asarray(results[c]["lncols"], dtype=np.float64).sum()
        sp_masked = -lns - (n_core - count) * ln2
        y_i = y[c * IMGS_PER_CORE : (c + 1) * IMGS_PER_CORE]
        num += sp_masked - (y_i * sxm).sum()
        cnt += count
    return np.float32(num / max(cnt, 1.0))


def kernel(cancer_logits, label, prostate_mask, needle_mask):
    nc = _get_nc()
    in_maps = _make_in_maps(cancer_logits, prostate_mask, needle_mask)
    res = run_bass_kernel_spmd(nc, in_maps, core_ids=list(range(N_CORES)))
    return _combine(res.results, label)


# revision 8
# speedup vs baseline: 1.0563x; 1.0563x over previous
"""Masked-BCE valid-region loss on 8 Trainium2 NeuronCores.

Inputs (full): cancer_logits [32,1,512,512] f32, label [32] f32,
prostate_mask [32,1,512,512] f32, needle_mask [32,1,512,512] f32.
Output: scalar f32 loss.

Sharding: data-parallel over batch - 4 images per core. The host packs
all three tensors as bf16 [128, 8192] per core (6 MB vs 12 MB f32).
DMA-engine busy is bound by the SBUF write side, so uploading bf16
directly over HWDGE beats fp8-in-HBM + SWDGE cast-DMA (same write
bytes, no Q7 descriptor-gen serialization).

Math: with m = (min(p,n) > 0.5) and y constant per image,

    bce = softplus(x) - x*y,   softplus(z) = -ln(sigmoid(-z))
    sum_masked softplus(x) = -sum ln(sigmoid(-xm)) - (N - count)*ln2

since masked-out elements have xm = 0 and contribute sigmoid(0) = 1/2.
ln of the 1M-element product is taken per group of 4 via a bf16
multiply tree (sigma <= 1 so products only shrink; worst case 4*6.9
lns stays far above bf16 underflow), so ACT runs ONE full-size pass
(sigmoid) instead of exp+ln and the final ln touches N/4 elements.

Device pipeline per chunk:

    pb,nb,xb = bf16 DMAs (HWDGE)
    pb  = min(pb, nb)                  # DVE tensor_tensor, 2x at bf16
    m   = (pb > 0.5)                   # DVE tensor_scalar, 4x
    xm  = m * xb                       # DVE tensor_tensor, 2x
    s   = sigmoid(-xm)                 # ACT (the only full-size pass)
    s2/s4 = halve-multiply tree        # DVE 2x, contiguous halves
    cnt += ones' @ m;  sxm_img += ones' @ xm   # TensorE -> PSUM
    ...all chunks done...
    ln(s4all) with accum_out           # ACT, one table switch total

tensor_scalar cannot carry an accumulator (BIR verifier rejects it)
and Pool/GpSimd cannot run TensorScalarPtr at all, so count and the
per-image sum(x*m) ride TensorE ones-matmuls into PSUM banks; image
banks are evacuated as soon as their image completes so only the count
bank and the ln ride the serial tail. All trees land in one shared
s4all tile so the single Ln op cannot be scheduled between sigmoids
(which would thrash the two pinned ACT table sets).
"""

import sys

for _p in ("/opt/trn_rl_repo", "/root/.axon_site/_ro/trn_rl_repo"):
    if _p not in sys.path:
        sys.path.append(_p)

import ml_dtypes
import numpy as np

import concourse.bacc as bacc
import concourse.tile as tile
from concourse import mybir
from concourse.bass_utils import run_bass_kernel_spmd

B, H, W = 32, 512, 512
N_CORES = 8
IMGS_PER_CORE = B // N_CORES  # 4
P = 128
FD = (H * W) // P  # 2048 free-dim elements per partition per image
N_PER_IMG = H * W  # 262144
TOT_FD = IMGS_PER_CORE * FD  # 8192
# chunks: multiples of 512 (PE blocks); small edges for ramp and tail
CHUNK_FDS = [512, 1536, 2048, 2048, 1536, 512]
N_CHUNKS = len(CHUNK_FDS)
TREE_K = 4  # elements per ln group (2 tree levels)

_nc_cache = None


def _patch_act_tables():
    """Keep only {sigmoid_and_others, natural_log} activation sets so
    exactly one ACT_TABLE_LOAD is emitted per set."""
    import concourse.hw_specs as hw_specs

    if getattr(bacc, "_act_tables_patched", False):
        return
    orig = hw_specs.get_activation_tables

    def patched(module_arch):
        tables = orig(module_arch)
        keep = ("sigmoid_and_others", "natural_log")
        return {
            name: (funcs if name in keep else set())
            for name, funcs in tables.items()
        }

    bacc.get_activation_tables = patched
    bacc._act_tables_patched = True


def _build_bass():
    _patch_act_tables()
    f32 = mybir.dt.float32
    bf16 = mybir.dt.bfloat16
    nc = bacc.Bacc()
    p_d = nc.dram_tensor("pb", [P, TOT_FD], bf16, kind="ExternalInput")
    n_d = nc.dram_tensor("nb", [P, TOT_FD], bf16, kind="ExternalInput")
    x_d = nc.dram_tensor("xb", [P, TOT_FD], bf16, kind="ExternalInput")
    ln_o = nc.dram_tensor("lncols", [P, 1], f32, kind="ExternalOutput")
    # [count row | img0..img3 rows], 512 f32 each
    red_o = nc.dram_tensor("red", [1, 5 * 512], f32, kind="ExternalOutput")

    with tile.TileContext(nc) as tc:
        with (
            tc.tile_pool(name="io", bufs=3) as io_pool,
            tc.tile_pool(name="work", bufs=3) as work_pool,
            tc.tile_pool(name="keep", bufs=1) as keep_pool,
            tc.tile_pool(name="psum", bufs=1, space="PSUM") as psum_pool,
        ):
            ones1 = keep_pool.tile([P, 1], bf16)
            nc.vector.memset(ones1, 1.0)
            lncols = keep_pool.tile([P, 1], f32)
            red_sb = keep_pool.tile([1, 5 * 512], f32)
            # every chunk's tree lands in one shared tile, so the single Ln
            # op depends on ALL trees and cannot be scheduled between
            # sigmoids (which would thrash the two ACT table sets).
            s4all = keep_pool.tile([P, TOT_FD // TREE_K], bf16)
            cnt_ps = psum_pool.tile([1, 512], f32, tag="cnt")
            img_ps = [
                psum_pool.tile([1, 512], f32, tag=f"img{i}", name=f"img_ps{i}")
                for i in range(IMGS_PER_CORE)
            ]

            total_blocks = TOT_FD // 512
            blocks_per_img = FD // 512
            cnt_done = 0
            img_done = [0] * IMGS_PER_CORE
            off = 0
            for c, cfd in enumerate(CHUNK_FDS):
                pb = io_pool.tile([P, cfd], bf16, tag="pb")
                nc.sync.dma_start(out=pb, in_=p_d[:, off : off + cfd])
                nb = io_pool.tile([P, cfd], bf16, tag="nb")
                nc.sync.dma_start(out=nb, in_=n_d[:, off : off + cfd])
                xb = io_pool.tile([P, cfd], bf16, tag="xb")
                nc.sync.dma_start(out=xb, in_=x_d[:, off : off + cfd])

                # pt = min(p, n), in place over pb
                nc.vector.tensor_tensor(
                    out=pb, in0=pb, in1=nb, op=mybir.AluOpType.min
                )
                m = work_pool.tile([P, cfd], bf16, tag="m")
                nc.vector.tensor_scalar(
                    out=m, in0=pb, scalar1=0.5, scalar2=None,
                    op0=mybir.AluOpType.is_gt,
                )
                xm = work_pool.tile([P, cfd], bf16, tag="xm")
                nc.vector.tensor_tensor(
                    out=xm, in0=m, in1=xb, op=mybir.AluOpType.mult
                )
                # s = sigmoid(-xm); bf16 out feeds the 2x multiply tree
                s = work_pool.tile([P, cfd], bf16, tag="s")
                nc.scalar.activation(
                    out=s, in_=xm, func=mybir.ActivationFunctionType.Sigmoid,
                    scale=-1.0,
                )
                h = cfd // 2
                s2 = work_pool.tile([P, h], bf16, tag="s2")
                nc.vector.tensor_tensor(
                    out=s2, in0=s[:, :h], in1=s[:, h:], op=mybir.AluOpType.mult
                )
                q = cfd // 4
                s4 = s4all[:, off // TREE_K : off // TREE_K + q]
                nc.vector.tensor_tensor(
                    out=s4, in0=s2[:, :q], in1=s2[:, q:], op=mybir.AluOpType.mult
                )

                # TensorE reductions: count (cnt bank) + per-image sum(xm)
                for s0 in range(0, cfd, 512):
                    nc.tensor.matmul(
                        cnt_ps, ones1, m[:, s0 : s0 + 512],
                        start=(cnt_done == 0),
                        stop=(cnt_done == total_blocks - 1),
                    )
                    cnt_done += 1
                    i = (off + s0) // FD
                    nc.tensor.matmul(
                        img_ps[i], ones1, xm[:, s0 : s0 + 512],
                        start=(img_done[i] == 0),
                        stop=(img_done[i] == blocks_per_img - 1),
                    )
                    img_done[i] += 1
                    if img_done[i] == blocks_per_img:
                        # evacuate this image's PSUM bank while later
                        # chunks still stream (keeps it off the tail)
                        nc.vector.tensor_scalar_add(
                            out=red_sb[:, (i + 1) * 512 : (i + 2) * 512],
                            in0=img_ps[i], scalar1=0.0,
                        )
                off += cfd

            # one ln over all trees: one table switch, one accum read
            lnv = keep_pool.tile([P, TOT_FD // TREE_K], bf16)
            nc.scalar.activation(
                out=lnv, in_=s4all, func=mybir.ActivationFunctionType.Ln,
                accum_out=lncols[:, 0:1],
            )
            nc.vector.tensor_scalar_add(
                out=red_sb[:, 0:512], in0=cnt_ps, scalar1=0.0
            )
            nc.sync.dma_start(out=ln_o[:], in_=lncols)
            nc.sync.dma_start(out=red_o[:], in_=red_sb)
    nc.finalize()
    return nc


def _get_nc():
    global _nc_cache
    if _nc_cache is None:
        _nc_cache = _build_bass()
    return _nc_cache


def _make_in_maps(cancer_logits, prostate_mask, needle_mask):
    bf = ml_dtypes.bfloat16

    # [B,1,H,W] -> [CORE, P, IMG*FD] image-major flat per-partition streams
    def pack(a):
        a = np.asarray(a, dtype=np.float32).reshape(B, P, FD).astype(bf)
        a = a.reshape(N_CORES, IMGS_PER_CORE, P, FD).transpose(0, 2, 1, 3)
        return np.ascontiguousarray(a).reshape(N_CORES, P, TOT_FD)

    xb = pack(cancer_logits)
    pb = pack(prostate_mask)
    nb = pack(needle_mask)
    return [
        {"pb": pb[c], "nb": nb[c], "xb": xb[c]} for c in range(N_CORES)
    ]


def _combine(results, label):
    y = np.asarray(label, dtype=np.float64).reshape(B)
    ln2 = float(np.log(2.0))
    n_core = IMGS_PER_CORE * N_PER_IMG
    num = 0.0
    cnt = 0.0
    for c in range(N_CORES):
        red = np.asarray(results[c]["red"], dtype=np.float64).reshape(5 * 512)
        count = red[:512].sum()
        sxm = red[512:].reshape(IMGS_PER_CORE, 512).sum(axis=1)
        lns = np.asarray(results[c]["lncols"], dtype=np.float64).sum()
        sp_masked = -lns - (n_core - count) * ln2
        y_i = y[c * IMGS_PER_CORE : (c + 1) * IMGS_PER_CORE]
        num += sp_masked - (y_i * sxm).sum()
        cnt += count
    return np.float32(num / max(cnt, 1.0))


def kernel(cancer_logits, label, prostate_mask, needle_mask):
    nc = _get_nc()
    in_maps = _make_in_maps(cancer_logits, prostate_mask, needle_mask)
    res = run_bass_kernel_spmd(nc, in_maps, core_ids=list(range(N_CORES)))
    return _combine(res.results, label)


# revision 9
# speedup vs baseline: 1.1932x; 1.1296x over previous
"""Masked-BCE valid-region loss on 8 Trainium2 NeuronCores.

Inputs (full): cancer_logits [32,1,512,512] f32, label [32] f32,
prostate_mask [32,1,512,512] f32, needle_mask [32,1,512,512] f32.
Output: scalar f32 loss.

Sharding: data-parallel over batch - 4 images per core. The host packs
all three tensors as bf16 into ONE stream per core, chunk-interleaved
as [p|n|x] blocks, so each chunk needs exactly one DMA and its compute
unblocks atomically (6 MB/core vs 12 MB f32; DMA-engine busy is bound
by the SBUF write side, so bf16-direct over HWDGE beats fp8+cast-DMA,
which writes the same bytes but serializes on Q7 descriptor-gen).

Math: with m = (min(p,n) > 0.5) and y constant per image,

    bce = softplus(x) - x*y
    sum_masked softplus(x) = sum ln(1 + e^xm) - (N - count)*ln2

since masked-out elements have xm = 0 and contribute ln2. The ln of
1M elements is folded as ln(prod) over groups of 4 via a bf16 multiply
tree ((1+e)^4 <= 2.4e11, far below bf16 overflow), so ACT runs one
full-size pass (exp) plus a quarter-size ln per chunk - both live in
the natural_log_exp_and_others table set, so there are no mid-kernel
ACT table switches and the lns pipeline with later chunks' exps.

Device pipeline per chunk (one [P, 3*cfd] tile holding p|n|x):

    pt  = min(p, n)                    # DVE tensor_tensor, 2x at bf16
    m   = (pt > 0.5)                   # DVE tensor_scalar, 4x
    xm  = m * x                        # DVE tensor_tensor, 2x
    e   = exp(xm)                      # ACT full-size pass
    e  += 1                            # DVE tensor_scalar, 4x, in place
    w2/w4 = halve-multiply tree        # DVE 2x, contiguous halves
    ln(w4) accum-> lncols[:, c]        # ACT quarter-size, interleaved
    cnt += ones' @ m;  sxm_img += ones' @ xm   # TensorE -> PSUM

tensor_scalar cannot carry an accumulator (BIR verifier rejects it)
and Pool/GpSimd cannot run TensorScalarPtr at all, so count and the
per-image sum(x*m) ride TensorE ones-matmuls into PSUM banks; image
banks are evacuated as soon as their image completes so only the count
bank and the last tiny ln ride the serial tail. The two output DMAs
are issued from different HWDGE engines (sync + scalar) to overlap
their completion latency.
"""

import sys

for _p in ("/opt/trn_rl_repo", "/root/.axon_site/_ro/trn_rl_repo"):
    if _p not in sys.path:
        sys.path.append(_p)

import ml_dtypes
import numpy as np

import concourse.bacc as bacc
import concourse.tile as tile
from concourse import mybir
from concourse.bass_utils import run_bass_kernel_spmd

B, H, W = 32, 512, 512
N_CORES = 8
IMGS_PER_CORE = B // N_CORES  # 4
P = 128
FD = (H * W) // P  # 2048 free-dim elements per partition per image
N_PER_IMG = H * W  # 262144
TOT_FD = IMGS_PER_CORE * FD  # 8192
# chunks: multiples of 512 (PE blocks); small first chunk for ramp,
# small last chunk for a short serial tail.
CHUNK_FDS = [512, 1024, 2048, 2048, 2048, 512]
N_CHUNKS = len(CHUNK_FDS)
TREE_K = 4  # elements per ln group (2 tree levels)

_nc_cache = None


def _patch_act_tables():
    """Pin every activation to natural_log_exp_and_others (exp + ln) so
    exactly one ACT_TABLE_LOAD is emitted."""
    import concourse.hw_specs as hw_specs

    if getattr(bacc, "_act_tables_patched", False):
        return
    orig = hw_specs.get_activation_tables

    def patched(module_arch):
        tables = orig(module_arch)
        keep = "natural_log_exp_and_others"
        if keep in tables:
            tables = {
                name: (funcs if name == keep else set())
                for name, funcs in tables.items()
            }
        return tables

    bacc.get_activation_tables = patched
    bacc._act_tables_patched = True


def _build_bass():
    _patch_act_tables()
    f32 = mybir.dt.float32
    bf16 = mybir.dt.bfloat16
    nc = bacc.Bacc()
    pnx_d = nc.dram_tensor("pnx", [P, 3 * TOT_FD], bf16, kind="ExternalInput")
    ln_o = nc.dram_tensor("lncols", [P, N_CHUNKS], f32, kind="ExternalOutput")
    # [count row | img0..img3 rows], 512 f32 each
    red_o = nc.dram_tensor("red", [1, 5 * 512], f32, kind="ExternalOutput")

    with tile.TileContext(nc) as tc:
        with (
            tc.tile_pool(name="io", bufs=3) as io_pool,
            tc.tile_pool(name="work", bufs=3) as work_pool,
            tc.tile_pool(name="keep", bufs=1) as keep_pool,
            tc.tile_pool(name="psum", bufs=1, space="PSUM") as psum_pool,
        ):
            ones1 = keep_pool.tile([P, 1], bf16)
            nc.vector.memset(ones1, 1.0)
            lncols = keep_pool.tile([P, N_CHUNKS], f32)
            red_sb = keep_pool.tile([1, 5 * 512], f32)
            cnt_ps = psum_pool.tile([1, 512], f32, tag="cnt")
            img_ps = [
                psum_pool.tile([1, 512], f32, tag=f"img{i}", name=f"img_ps{i}")
                for i in range(IMGS_PER_CORE)
            ]

            total_blocks = TOT_FD // 512
            blocks_per_img = FD // 512
            cnt_done = 0
            img_done = [0] * IMGS_PER_CORE
            off = 0
            for c, cfd in enumerate(CHUNK_FDS):
                pnx = io_pool.tile([P, 3 * cfd], bf16, tag="pnx")
                nc.sync.dma_start(
                    out=pnx, in_=pnx_d[:, 3 * off : 3 * off + 3 * cfd]
                )
                pb = pnx[:, :cfd]
                nb = pnx[:, cfd : 2 * cfd]
                xb = pnx[:, 2 * cfd :]

                # pt = min(p, n), in place over the p block
                nc.vector.tensor_tensor(
                    out=pb, in0=pb, in1=nb, op=mybir.AluOpType.min
                )
                m = work_pool.tile([P, cfd], bf16, tag="m")
                nc.vector.tensor_scalar(
                    out=m, in0=pb, scalar1=0.5, scalar2=None,
                    op0=mybir.AluOpType.is_gt,
                )
                xm = work_pool.tile([P, cfd], bf16, tag="xm")
                nc.vector.tensor_tensor(
                    out=xm, in0=m, in1=xb, op=mybir.AluOpType.mult
                )
                # w = exp(xm) + 1
                w = work_pool.tile([P, cfd], bf16, tag="w")
                nc.scalar.activation(
                    out=w, in_=xm, func=mybir.ActivationFunctionType.Exp,
                )
                nc.vector.tensor_scalar(
                    out=w, in0=w, scalar1=1.0, scalar2=None,
                    op0=mybir.AluOpType.add,
                )
                h = cfd // 2
                w2 = work_pool.tile([P, h], bf16, tag="w2")
                nc.vector.tensor_tensor(
                    out=w2, in0=w[:, :h], in1=w[:, h:], op=mybir.AluOpType.mult
                )
                q = cfd // 4
                w4 = work_pool.tile([P, q], bf16, tag="w4")
                nc.vector.tensor_tensor(
                    out=w4, in0=w2[:, :q], in1=w2[:, q:], op=mybir.AluOpType.mult
                )
                # ln of the chunk's products, accumulated per partition;
                # exp and ln share one table set so this pipelines freely.
                lnv = work_pool.tile([P, q], bf16, tag="lnv")
                nc.scalar.activation(
                    out=lnv, in_=w4, func=mybir.ActivationFunctionType.Ln,
                    accum_out=lncols[:, c : c + 1],
                )

                # TensorE reductions: count (cnt bank) + per-image sum(xm)
                for s0 in range(0, cfd, 512):
                    nc.tensor.matmul(
                        cnt_ps, ones1, m[:, s0 : s0 + 512],
                        start=(cnt_done == 0),
                        stop=(cnt_done == total_blocks - 1),
                    )
                    cnt_done += 1
                    i = (off + s0) // FD
                    nc.tensor.matmul(
                        img_ps[i], ones1, xm[:, s0 : s0 + 512],
                        start=(img_done[i] == 0),
                        stop=(img_done[i] == blocks_per_img - 1),
                    )
                    img_done[i] += 1
                    if img_done[i] == blocks_per_img:
                        # evacuate this image's PSUM bank while later
                        # chunks still stream (keeps it off the tail)
                        nc.vector.tensor_scalar_add(
                            out=red_sb[:, (i + 1) * 512 : (i + 2) * 512],
                            in0=img_ps[i], scalar1=0.0,
                        )
                off += cfd

            nc.vector.tensor_scalar_add(
                out=red_sb[:, 0:512], in0=cnt_ps, scalar1=0.0
            )
            # parallel completion: lncols via the scalar HWDGE ring,
            # red via sync
            nc.scalar.dma_start(out=ln_o[:], in_=lncols)
            nc.sync.dma_start(out=red_o[:], in_=red_sb)
    nc.finalize()
    return nc


def _get_nc():
    global _nc_cache
    if _nc_cache is None:
        _nc_cache = _build_bass()
    return _nc_cache


# global col offsets of each chunk
_CHUNK_OFFS = []
_off = 0
for _cfd in CHUNK_FDS:
    _CHUNK_OFFS.append(_off)
    _off += _cfd


def _make_in_maps(cancer_logits, prostate_mask, needle_mask):
    bf = ml_dtypes.bfloat16

    # [B,1,H,W] -> [CORE, P, IMG*FD] image-major flat per-partition streams
    def pack(a):
        a = np.asarray(a, dtype=np.float32).reshape(B, P, FD).astype(bf)
        a = a.reshape(N_CORES, IMGS_PER_CORE, P, FD).transpose(0, 2, 1, 3)
        return np.ascontiguousarray(a).reshape(N_CORES, P, TOT_FD)

    xb = pack(cancer_logits)
    pb = pack(prostate_mask)
    nb = pack(needle_mask)
    pnx = np.empty((N_CORES, P, 3 * TOT_FD), dtype=bf)
    for off, cfd in zip(_CHUNK_OFFS, CHUNK_FDS):
        o3 = 3 * off
        pnx[:, :, o3 : o3 + cfd] = pb[:, :, off : off + cfd]
        pnx[:, :, o3 + cfd : o3 + 2 * cfd] = nb[:, :, off : off + cfd]
        pnx[:, :, o3 + 2 * cfd : o3 + 3 * cfd] = xb[:, :, off : off + cfd]
    return [{"pnx": pnx[c]} for c in range(N_CORES)]


def _combine(results, label):
    y = np.asarray(label, dtype=np.float64).reshape(B)
    ln2 = float(np.log(2.0))
    n_core = IMGS_PER_CORE * N_PER_IMG
    num = 0.0
    cnt = 0.0
    for c in range(N_CORES):
        red = np.asarray(results[c]["red"], dtype=np.float64).reshape(5 * 512)
        count = red[:512].sum()
        sxm = red[512:].reshape(IMGS_PER_CORE, 512).sum(axis=1)
        lns = np.asarray(results[c]["lncols"], dtype=np.float64).sum()
        sp_masked = lns - (n_core - count) * ln2
        y_i = y[c * IMGS_PER_CORE : (c + 1) * IMGS_PER_CORE]
        num += sp_masked - (y_i * sxm).sum()
        cnt += count
    return np.float32(num / max(cnt, 1.0))


def kernel(cancer_logits, label, prostate_mask, needle_mask):
    nc = _get_nc()
    in_maps = _make_in_maps(cancer_logits, prostate_mask, needle_mask)
    res = run_bass_kernel_spmd(nc, in_maps, core_ids=list(range(N_CORES)))
    return _combine(res.results, label)


# revision 10
# speedup vs baseline: 1.2912x; 1.0821x over previous
"""Masked-BCE valid-region loss on 8 Trainium2 NeuronCores.

Inputs (full): cancer_logits [32,1,512,512] f32, label [32] f32,
prostate_mask [32,1,512,512] f32, needle_mask [32,1,512,512] f32.
Output: scalar f32 loss.

Sharding: data-parallel over batch - 4 images per core. Per-core HBM
streams: masks as ONE chunk-blocked [p|n] fp8e4m3 stream (2 MB) and
logits as bf16 (2 MB) - 4 MB/core vs 12 MB f32. Mask threshold flips
from fp8 rounding hit numerator and denominator on the same pixels,
so the loss ratio moves only ~1e-4.

Math: with m = (min(p,n) > 0.5) and y constant per image,

    bce = softplus(x) - x*y
    sum_masked softplus(x) = sum ln(1 + e^xm) - (N - count)*ln2

since masked-out elements have xm = 0 and contribute ln2. The ln of
1M elements is folded as ln(prod) over groups of 4 via a bf16 multiply
tree ((1+e)^4 <= 2.4e11, far below bf16 overflow), so ACT runs one
full-size pass (exp) plus a quarter-size ln per chunk - both live in
the natural_log_exp_and_others table set, so there are no mid-kernel
ACT table switches and the lns pipeline with later chunks' exps.

Two custom DVE ops (registered into dve_ops at import, table bytes are
embedded in the HLO at trace time):

    MASK_MIN_GT_CNT_ANT: out = (min(p8,n8) > 0.5), accum = count.
        Custom ops run at 1x for any dtype, so the masks can stay fp8
        (halving their DMA cost) while min+compare+count fuse into one
        pass - stock ops would need a bf16 cast-DMA (write-side bound),
        two DVE passes, and a TensorE count reduction.
    TREE_MUL_P1_ANT: out = (a+1)*(b+1) - folds the exp+1 into the
        first tree level.

Device pipeline per chunk:

    m   = (min(p8,n8) > 0.5), count     # custom DVE, fp8 in
    xm  = m * x                         # DVE tensor_tensor, 2x bf16
    e   = exp(xm)                       # ACT full-size pass
    w2  = (e_lo+1)*(e_hi+1)             # custom DVE
    w4  = w2_lo * w2_hi                 # DVE 2x
    ln(w4) accum -> lncols[:, c]        # ACT quarter-size, interleaved
    sxm_img += ones' @ xm               # TensorE -> PSUM (evacuated
                                        #  as each image completes)

tensor_scalar cannot carry an accumulator (BIR verifier rejects it)
and Pool/GpSimd cannot run TensorScalarPtr at all, which is why the
per-image sum(x*m) rides TensorE ones-matmuls. The two output DMAs go
out on different HWDGE engines (sync + scalar) to overlap completion.
"""

import sys

for _p in ("/opt/trn_rl_repo", "/root/.axon_site/_ro/trn_rl_repo"):
    if _p not in sys.path:
        sys.path.append(_p)

import ml_dtypes
import numpy as np

import concourse.bacc as bacc
import concourse.tile as tile
from concourse import mybir
from concourse.bass_utils import run_bass_kernel_spmd

B, H, W = 32, 512, 512
N_CORES = 8
IMGS_PER_CORE = B // N_CORES  # 4
P = 128
FD = (H * W) // P  # 2048 free-dim elements per partition per image
N_PER_IMG = H * W  # 262144
TOT_FD = IMGS_PER_CORE * FD  # 8192
# chunks: multiples of 512 (PE blocks); small first chunk for ramp,
# small last chunk for a short serial tail.
CHUNK_FDS = [512, 1024, 2048, 2048, 2048, 512]
N_CHUNKS = len(CHUNK_FDS)

_nc_cache = None


def _patch_act_tables():
    """Pin every activation to natural_log_exp_and_others (exp + ln) so
    exactly one ACT_TABLE_LOAD is emitted."""
    import concourse.hw_specs as hw_specs

    if getattr(bacc, "_act_tables_patched", False):
        return
    orig = hw_specs.get_activation_tables

    def patched(module_arch):
        tables = orig(module_arch)
        keep = "natural_log_exp_and_others"
        if keep in tables:
            tables = {
                name: (funcs if name == keep else set())
                for name, funcs in tables.items()
            }
        return tables

    bacc.get_activation_tables = patched
    bacc._act_tables_patched = True


def _register_custom_ops():
    """Register the two fused DVE ops into dve_ops' tables. The uops_sha
    pin is computed from lower() here, so it is self-consistent by
    construction; correctness is asserted against numpy by the test."""
    import concourse.dve_ops as dvo
    from concourse.dve_spec import AluOp, C0, One, Spec, Src0, Src1, lower, minn
    from concourse.dve_uop import DveOpSpec

    if hasattr(dvo, "MASK_MIN_GT_CNT_ANT"):
        return

    def mask_ref(in0, in1, c0, c1, c2):
        m = (np.minimum(in0, in1) > c0).astype(np.float32)
        return m, m.sum(axis=1, keepdims=True)

    specs = [
        (
            "MASK_MIN_GT_CNT_ANT",
            Spec(body=(minn(Src0, Src1) > C0), accum=AluOp.ADD, reference=mask_ref),
        ),
        (
            "TREE_MUL_P1_ANT",
            Spec(
                body=(Src0 + One) * (Src1 + One),
                reference=lambda in0, in1, c0, c1, c2: (in0 + 1.0) * (in1 + 1.0),
            ),
        ),
    ]
    for name, spec in specs:
        row = dvo._CUSTOM_DVE_ROW_BASE + len(dvo.OPS)
        shas = {}
        for ver in ("v3", "v4"):
            s = DveOpSpec(name=name, opcode=row, uops=lower(spec, ver=ver), rd1_en=True)
            shas[ver] = s.sha(ver)
        op = dvo.DveOp(name, spec, subdim=False, uops_sha=shas)
        dvo.OPS.append(op)
        dvo._SUB_OPCODE_FOR_NAME[name] = row
        dvo.CUSTOM_DVE_SPECS[name] = spec
        setattr(dvo, name, op)


def _build_bass():
    _patch_act_tables()
    _register_custom_ops()
    import concourse.dve_ops as dvo

    f32 = mybir.dt.float32
    bf16 = mybir.dt.bfloat16
    fp8 = mybir.dt.float8e4
    nc = bacc.Bacc()
    pn_d = nc.dram_tensor("pn8", [P, 2 * TOT_FD], fp8, kind="ExternalInput")
    x_d = nc.dram_tensor("xb", [P, TOT_FD], bf16, kind="ExternalInput")
    # [count col per chunk | ln col per chunk]
    cols_o = nc.dram_tensor("cols", [P, 2 * N_CHUNKS], f32, kind="ExternalOutput")
    # img0..img3 rows, 512 f32 each
    red_o = nc.dram_tensor("red", [1, 4 * 512], f32, kind="ExternalOutput")

    with tile.TileContext(nc) as tc:
        with (
            tc.tile_pool(name="io", bufs=3) as io_pool,
            tc.tile_pool(name="work", bufs=3) as work_pool,
            tc.tile_pool(name="keep", bufs=1) as keep_pool,
            tc.tile_pool(name="psum", bufs=1, space="PSUM") as psum_pool,
        ):
            ones1 = keep_pool.tile([P, 1], bf16)
            nc.vector.memset(ones1, 1.0)
            cols = keep_pool.tile([P, 2 * N_CHUNKS], f32)
            red_sb = keep_pool.tile([1, 4 * 512], f32)
            img_ps = [
                psum_pool.tile([1, 512], f32, tag=f"img{i}", name=f"img_ps{i}")
                for i in range(IMGS_PER_CORE)
            ]

            blocks_per_img = FD // 512
            img_done = [0] * IMGS_PER_CORE
            off = 0
            for c, cfd in enumerate(CHUNK_FDS):
                pn = io_pool.tile([P, 2 * cfd], fp8, tag="pn")
                nc.sync.dma_start(out=pn, in_=pn_d[:, 2 * off : 2 * off + 2 * cfd])
                xb = io_pool.tile([P, cfd], bf16, tag="xb")
                nc.sync.dma_start(out=xb, in_=x_d[:, off : off + cfd])

                # m = (min(p, n) > 0.5) with fused count accumulation
                m = work_pool.tile([P, cfd], bf16, tag="m")
                nc.vector._custom_dve(
                    dvo.MASK_MIN_GT_CNT_ANT,
                    out=m, in0=pn[:, :cfd], in1=pn[:, cfd:],
                    s0=0.5, accum_out=cols[:, c : c + 1],
                )
                xm = work_pool.tile([P, cfd], bf16, tag="xm")
                nc.vector.tensor_tensor(
                    out=xm, in0=m, in1=xb, op=mybir.AluOpType.mult
                )
                # e = exp(xm); tree folds the +1 into its first level
                e = work_pool.tile([P, cfd], bf16, tag="e")
                nc.scalar.activation(
                    out=e, in_=xm, func=mybir.ActivationFunctionType.Exp,
                )
                h = cfd // 2
                w2 = work_pool.tile([P, h], bf16, tag="w2")
                nc.vector._custom_dve(
                    dvo.TREE_MUL_P1_ANT, out=w2, in0=e[:, :h], in1=e[:, h:]
                )
                q = cfd // 4
                w4 = work_pool.tile([P, q], bf16, tag="w4")
                nc.vector.tensor_tensor(
                    out=w4, in0=w2[:, :q], in1=w2[:, q:], op=mybir.AluOpType.mult
                )
                # ln of the chunk's products, accumulated per partition;
                # exp and ln share one table set so this pipelines freely.
                lnv = work_pool.tile([P, q], bf16, tag="lnv")
                nc.scalar.activation(
                    out=lnv, in_=w4, func=mybir.ActivationFunctionType.Ln,
                    accum_out=cols[:, N_CHUNKS + c : N_CHUNKS + c + 1],
                )

                # TensorE: per-image sum(xm)
                for s0 in range(0, cfd, 512):
                    i = (off + s0) // FD
                    nc.tensor.matmul(
                        img_ps[i], ones1, xm[:, s0 : s0 + 512],
                        start=(img_done[i] == 0),
                        stop=(img_done[i] == blocks_per_img - 1),
                    )
                    img_done[i] += 1
                    if img_done[i] == blocks_per_img:
                        # evacuate this image's PSUM bank while later
                        # chunks still stream (keeps it off the tail)
                        nc.vector.tensor_scalar_add(
                            out=red_sb[:, i * 512 : (i + 1) * 512],
                            in0=img_ps[i], scalar1=0.0,
                        )
                off += cfd

            # parallel completion: cols via the scalar HWDGE ring, red via sync
            nc.scalar.dma_start(out=cols_o[:], in_=cols)
            nc.sync.dma_start(out=red_o[:], in_=red_sb)
    nc.finalize()
    return nc


def _get_nc():
    global _nc_cache
    if _nc_cache is None:
        _nc_cache = _build_bass()
    return _nc_cache


# global col offsets of each chunk
_CHUNK_OFFS = []
_off = 0
for _cfd in CHUNK_FDS:
    _CHUNK_OFFS.append(_off)
    _off += _cfd


def _make_in_maps(cancer_logits, prostate_mask, needle_mask):
    bf = ml_dtypes.bfloat16
    f8 = ml_dtypes.float8_e4m3

    # [B,1,H,W] -> [CORE, P, IMG*FD] image-major flat per-partition streams
    def pack(a, dt):
        a = np.asarray(a, dtype=np.float32).reshape(B, P, FD).astype(dt)
        a = a.reshape(N_CORES, IMGS_PER_CORE, P, FD).transpose(0, 2, 1, 3)
        return np.ascontiguousarray(a).reshape(N_CORES, P, TOT_FD)

    xb = pack(cancer_logits, bf)
    pb = pack(prostate_mask, f8)
    nb = pack(needle_mask, f8)
    pn = np.empty((N_CORES, P, 2 * TOT_FD), dtype=f8)
    for off, cfd in zip(_CHUNK_OFFS, CHUNK_FDS):
        o2 = 2 * off
        pn[:, :, o2 : o2 + cfd] = pb[:, :, off : off + cfd]
        pn[:, :, o2 + cfd : o2 + 2 * cfd] = nb[:, :, off : off + cfd]
    return [{"pn8": pn[c], "xb": xb[c]} for c in range(N_CORES)]


def _combine(results, label):
    y = np.asarray(label, dtype=np.float64).reshape(B)
    ln2 = float(np.log(2.0))
    n_core = IMGS_PER_CORE * N_PER_IMG
    num = 0.0
    cnt = 0.0
    for c in range(N_CORES):
        red = np.asarray(results[c]["red"], dtype=np.float64).reshape(4 * 512)
        sxm = red.reshape(IMGS_PER_CORE, 512).sum(axis=1)
        cols = np.asarray(results[c]["cols"], dtype=np.float64)
        count = cols[:, :N_CHUNKS].sum()
        lns = cols[:, N_CHUNKS:].sum()
        sp_masked = lns - (n_core - count) * ln2
        y_i = y[c * IMGS_PER_CORE : (c + 1) * IMGS_PER_CORE]
        num += sp_masked - (y_i * sxm).sum()
        cnt += count
    return np.float32(num / max(cnt, 1.0))


def kernel(cancer_logits, label, prostate_mask, needle_mask):
    nc = _get_nc()
    in_maps = _make_in_maps(cancer_logits, prostate_mask, needle_mask)
    res = run_bass_kernel_spmd(nc, in_maps, core_ids=list(range(N_CORES)))
    return _combine(res.results, label)


# revision 14
# speedup vs baseline: 1.3818x; 1.0702x over previous
"""Masked-BCE valid-region loss on 8 Trainium2 NeuronCores.

Inputs (full): cancer_logits [32,1,512,512] f32, label [32] f32,
prostate_mask [32,1,512,512] f32, needle_mask [32,1,512,512] f32.
Output: scalar f32 loss.

Sharding: data-parallel over batch - 4 images per core. Per-core HBM
streams: masks as ONE chunk-blocked [p|n] fp8e4m3 stream (2 MB) and
logits as bf16 (2 MB) - 4 MB/core vs 12 MB f32. Mask threshold flips
from fp8 rounding hit numerator and denominator on the same pixels,
so the loss ratio moves only ~1e-4.

Math: with m = (min(p,n) > 0.5) and y constant per image,

    bce = softplus(x) - x*y
    sum_masked softplus(x) = sum ln(1 + e^xm) - (N - count)*ln2

since masked-out elements have xm = 0 and contribute ln2. The ln of
1M elements is folded as ln(prod) over groups of 4 via a bf16 multiply
tree ((1+e)^4 <= 2.4e11, far below bf16 overflow), so ACT runs one
full-size pass (exp) plus a quarter-size ln per chunk - both live in
the natural_log_exp_and_others table set, so there are no mid-kernel
ACT table switches and the lns pipeline with later chunks' exps.

Two custom DVE ops (registered into dve_ops at import, table bytes are
embedded in the HLO at trace time):

    MASK_MIN_GT_CNT_ANT: out = (min(p8,n8) > 0.5), accum = count.
        Custom ops run at 1x for any dtype, so the masks can stay fp8
        (halving their DMA cost) while min+compare+count fuse into one
        pass - stock ops would need a bf16 cast-DMA (write-side bound),
        two DVE passes, and a TensorE count reduction.
    TREE_MUL_P1_ANT: out = (a+1)*(b+1) - folds the exp+1 into the
        first tree level.

Device pipeline per chunk:

    m   = (min(p8,n8) > 0.5), count     # custom DVE, fp8 in
    xm  = m * x                         # DVE tensor_tensor, 2x bf16
    e   = exp(xm)                       # ACT full-size pass
    w2  = (e_lo+1)*(e_hi+1)             # custom DVE
    w4  = w2_lo * w2_hi                 # DVE 2x
    ln(w4) accum -> lncols[:, c]        # ACT quarter-size, interleaved
    sxm_img += ones' @ xm               # TensorE -> PSUM (evacuated
                                        #  as each image completes)

tensor_scalar cannot carry an accumulator (BIR verifier rejects it)
and Pool/GpSimd cannot run TensorScalarPtr at all, which is why the
per-image sum(x*m) rides TensorE ones-matmuls. The two output DMAs go
out on different HWDGE engines (sync + scalar) to overlap completion.
"""

import sys

for _p in ("/opt/trn_rl_repo", "/root/.axon_site/_ro/trn_rl_repo"):
    if _p not in sys.path:
        sys.path.append(_p)

import ml_dtypes
import numpy as np

import concourse.bacc as bacc
import concourse.tile as tile
from concourse import mybir
from concourse.bass_utils import run_bass_kernel_spmd

B, H, W = 32, 512, 512
N_CORES = 8
IMGS_PER_CORE = B // N_CORES  # 4
P = 128
FD = (H * W) // P  # 2048 free-dim elements per partition per image
N_PER_IMG = H * W  # 262144
TOT_FD = IMGS_PER_CORE * FD  # 8192
# chunks: multiples of 512 (PE blocks); small first chunk for ramp,
# small last chunk for a short serial tail.
CHUNK_FDS = [512, 1536, 2048, 2048, 1536, 512]
N_CHUNKS = len(CHUNK_FDS)

_nc_cache = None


def _patch_act_tables():
    """Pin every activation to natural_log_exp_and_others (exp + ln) so
    exactly one ACT_TABLE_LOAD is emitted."""
    import concourse.hw_specs as hw_specs

    if getattr(bacc, "_act_tables_patched", False):
        return
    orig = hw_specs.get_activation_tables

    def patched(module_arch):
        tables = orig(module_arch)
        keep = "natural_log_exp_and_others"
        if keep in tables:
            tables = {
                name: (funcs if name == keep else set())
                for name, funcs in tables.items()
            }
        return tables

    bacc.get_activation_tables = patched
    bacc._act_tables_patched = True


def _register_custom_ops():
    """Register the two fused DVE ops into dve_ops' tables. The uops_sha
    pin is computed from lower() here, so it is self-consistent by
    construction; correctness is asserted against numpy by the test."""
    import concourse.dve_ops as dvo
    from concourse.dve_spec import AluOp, C0, One, Spec, Src0, Src1, lower, minn
    from concourse.dve_uop import DveOpSpec

    if hasattr(dvo, "MASK_MIN_GT_CNT_ANT"):
        return

    def mask_ref(in0, in1, c0, c1, c2):
        m = (np.minimum(in0, in1) > c0).astype(np.float32)
        return m, m.sum(axis=1, keepdims=True)

    specs = [
        (
            "MASK_MIN_GT_CNT_ANT",
            Spec(body=(minn(Src0, Src1) > C0), accum=AluOp.ADD, reference=mask_ref),
        ),
        (
            "TREE_MUL_P1_ANT",
            Spec(
                body=(Src0 + One) * (Src1 + One),
                reference=lambda in0, in1, c0, c1, c2: (in0 + 1.0) * (in1 + 1.0),
            ),
        ),
    ]
    for name, spec in specs:
        row = dvo._CUSTOM_DVE_ROW_BASE + len(dvo.OPS)
        shas = {}
        for ver in ("v3", "v4"):
            s = DveOpSpec(name=name, opcode=row, uops=lower(spec, ver=ver), rd1_en=True)
            shas[ver] = s.sha(ver)
        op = dvo.DveOp(name, spec, subdim=False, uops_sha=shas)
        dvo.OPS.append(op)
        dvo._SUB_OPCODE_FOR_NAME[name] = row
        dvo.CUSTOM_DVE_SPECS[name] = spec
        setattr(dvo, name, op)


def _build_bass():
    _patch_act_tables()
    _register_custom_ops()
    import concourse.dve_ops as dvo

    f32 = mybir.dt.float32
    bf16 = mybir.dt.bfloat16
    fp8 = mybir.dt.float8e4
    nc = bacc.Bacc()
    pn_d = nc.dram_tensor("pn8", [P, 2 * TOT_FD], fp8, kind="ExternalInput")
    x_d = nc.dram_tensor("xb", [P, TOT_FD], bf16, kind="ExternalInput")
    # [count col per chunk | ln col per chunk]
    cols_o = nc.dram_tensor("cols", [P, 2 * N_CHUNKS], f32, kind="ExternalOutput")
    # img0..img3 rows, 512 f32 each
    red_o = nc.dram_tensor("red", [1, 4 * 512], f32, kind="ExternalOutput")

    with tile.TileContext(nc) as tc:
        with (
            tc.tile_pool(name="io", bufs=3) as io_pool,
            tc.tile_pool(name="work", bufs=3) as work_pool,
            tc.tile_pool(name="keep", bufs=1) as keep_pool,
            tc.tile_pool(name="psum", bufs=1, space="PSUM") as psum_pool,
        ):
            ones1 = keep_pool.tile([P, 1], bf16)
            nc.vector.memset(ones1, 1.0)
            cols = keep_pool.tile([P, 2 * N_CHUNKS], f32)
            red_sb = keep_pool.tile([1, 4 * 512], f32)
            img_ps = [
                psum_pool.tile([1, 512], f32, tag=f"img{i}", name=f"img_ps{i}")
                for i in range(IMGS_PER_CORE)
            ]

            blocks_per_img = FD // 512
            img_done = [0] * IMGS_PER_CORE
            off = 0
            for c, cfd in enumerate(CHUNK_FDS):
                pn = io_pool.tile([P, 2 * cfd], fp8, tag="pn")
                nc.sync.dma_start(out=pn, in_=pn_d[:, 2 * off : 2 * off + 2 * cfd])
                xb = io_pool.tile([P, cfd], bf16, tag="xb")
                nc.sync.dma_start(out=xb, in_=x_d[:, off : off + cfd])

                # m = (min(p, n) > 0.5) with fused count accumulation
                m = work_pool.tile([P, cfd], bf16, tag="m")
                nc.vector._custom_dve(
                    dvo.MASK_MIN_GT_CNT_ANT,
                    out=m, in0=pn[:, :cfd], in1=pn[:, cfd:],
                    s0=0.5, accum_out=cols[:, c : c + 1],
                )
                xm = work_pool.tile([P, cfd], bf16, tag="xm")
                nc.vector.tensor_tensor(
                    out=xm, in0=m, in1=xb, op=mybir.AluOpType.mult
                )
                # e = exp(xm); tree folds the +1 into its first level
                e = work_pool.tile([P, cfd], bf16, tag="e")
                nc.scalar.activation(
                    out=e, in_=xm, func=mybir.ActivationFunctionType.Exp,
                )
                h = cfd // 2
                w2 = work_pool.tile([P, h], bf16, tag="w2")
                nc.vector._custom_dve(
                    dvo.TREE_MUL_P1_ANT, out=w2, in0=e[:, :h], in1=e[:, h:]
                )
                q = cfd // 4
                w4 = work_pool.tile([P, q], bf16, tag="w4")
                nc.vector.tensor_tensor(
                    out=w4, in0=w2[:, :q], in1=w2[:, q:], op=mybir.AluOpType.mult
                )
                # ln of the chunk's products, accumulated per partition;
                # exp and ln share one table set so this pipelines freely.
                lnv = work_pool.tile([P, q], bf16, tag="lnv")
                nc.scalar.activation(
                    out=lnv, in_=w4, func=mybir.ActivationFunctionType.Ln,
                    accum_out=cols[:, N_CHUNKS + c : N_CHUNKS + c + 1],
                )

                # TensorE: per-image sum(xm)
                for s0 in range(0, cfd, 512):
                    i = (off + s0) // FD
                    nc.tensor.matmul(
                        img_ps[i], ones1, xm[:, s0 : s0 + 512],
                        start=(img_done[i] == 0),
                        stop=(img_done[i] == blocks_per_img - 1),
                    )
                    img_done[i] += 1
                    if img_done[i] == blocks_per_img:
                        # evacuate this image's PSUM bank while later
                        # chunks still stream (keeps it off the tail);
                        # ACT sits closest to PSUM and spares the DVE
                        # (GpSimd cannot access PSUM at all)
                        nc.scalar.copy(
                            out=red_sb[:, i * 512 : (i + 1) * 512],
                            in_=img_ps[i],
                        )
                off += cfd

            # parallel completion: cols via the scalar HWDGE ring, red via sync
            nc.scalar.dma_start(out=cols_o[:], in_=cols)
            nc.sync.dma_start(out=red_o[:], in_=red_sb)
    nc.finalize()
    return nc


def _get_nc():
    global _nc_cache
    if _nc_cache is None:
        _nc_cache = _build_bass()
    return _nc_cache


# global col offsets of each chunk
_CHUNK_OFFS = []
_off = 0
for _cfd in CHUNK_FDS:
    _CHUNK_OFFS.append(_off)
    _off += _cfd


def _make_in_maps(cancer_logits, prostate_mask, needle_mask):
    bf = ml_dtypes.bfloat16
    f8 = ml_dtypes.float8_e4m3

    # [B,1,H,W] -> [CORE, P, IMG*FD] image-major flat per-partition streams
    def pack(a, dt):
        a = np.asarray(a, dtype=np.float32).reshape(B, P, FD).astype(dt)
        a = a.reshape(N_CORES, IMGS_PER_CORE, P, FD).transpose(0, 2, 1, 3)
        return np.ascontiguousarray(a).reshape(N_CORES, P, TOT_FD)

    xb = pack(cancer_logits, bf)
    pb = pack(prostate_mask, f8)
    nb = pack(needle_mask, f8)
    pn = np.empty((N_CORES, P, 2 * TOT_FD), dtype=f8)
    for off, cfd in zip(_CHUNK_OFFS, CHUNK_FDS):
        o2 = 2 * off
        pn[:, :, o2 : o2 + cfd] = pb[:, :, off : off + cfd]
        pn[:, :, o2 + cfd : o2 + 2 * cfd] = nb[:, :, off : off + cfd]
    return [{"pn8": pn[c], "xb": xb[c]} for c in range(N_CORES)]


def _combine(results, label):
    y = np.asarray(label, dtype=np.float64).reshape(B)
    ln2 = float(np.log(2.0))
    n_core = IMGS_PER_CORE * N_PER_IMG
    num = 0.0
    cnt = 0.0
    for c in range(N_CORES):
        red = np.asarray(results[c]["red"], dtype=np.float64).reshape(4 * 512)
        sxm = red.reshape(IMGS_PER_CORE, 512).sum(axis=1)
        cols = np.asarray(results[c]["cols"], dtype=np.float64)
        count = cols[:, :N_CHUNKS].sum()
        lns = cols[:, N_CHUNKS:].sum()
        sp_masked = lns - (n_core - count) * ln2
        y_i = y[c * IMGS_PER_CORE : (c + 1) * IMGS_PER_CORE]
        num += sp_masked - (y_i * sxm).sum()
        cnt += count
    return np.float32(num / max(cnt, 1.0))


def kernel(cancer_logits, label, prostate_mask, needle_mask):
    nc = _get_nc()
    in_maps = _make_in_maps(cancer_logits, prostate_mask, needle_mask)
    res = run_bass_kernel_spmd(nc, in_maps, core_ids=list(range(N_CORES)))
    return _combine(res.results, label)
